# revision 9
# baseline (speedup 1.0000x reference)
"""Trainium2 Bass kernel for nn_Attention3D (RMSNorm3D + 1x1x1 QKV conv +
4-head non-flash attention over n=4096 tokens + 1x1x1 output conv).

Sharding: b*heads = 2*4 = 8 independent attention instances -> one per
NeuronCore. Each core computes its (batch, head) attention and the partial
output projection partial_h = w_out[:, head] @ attn_out_h, returned
token-major [4096, 64]. Host sums the 4 head partials per batch and adds
the bias.

Per-core pipeline (all shapes per core):
  x_b [64, 4096] --(x^2, ones-matmul)--> sumsq per token -> inv = 1/max(l2,eps)
  qkv chunks: lhsT=x[:,128-chunk] @ w_qkv' [64,96] -> [128t, 96] PSUM
     (w' has g*sqrt(C) folded; q also has dh^-0.5)
     * inv[t] per-partition -> qkv_sb bf16 [128, 32, 97] (col 96 = ones)
  qT/kT 128x32 blocks are PE-transposed into row-layout q4 / k4 [32, 4096]
  scores (transposed): simT[j, i-block] = k_chunk.T @ q  (K=dh=32)
  e = exp(simT) on ScalarE (psum -> sbuf, bf16, no max-subtraction needed:
      |sim| <= ~1.2 for this problem's data)
  PV: lhsT = [v | ones] [128j, 33] -> out [33, i] accumulated over j chunks;
      row 32 = softmax denominator. 2-way column tiling.
  out-proj: woT.T @ num -> [64, i] psum; PE-transpose 128-chunks ->
      [128t, 64], * (1/s)[t] -> out_tok [4096, 64] f32 -> HBM
"""

import os

import numpy as np

import concourse.bass as bass
import concourse.mybir as mybir
import concourse.tile as tile
from concourse import bacc
from concourse.bass import ts
from concourse.bass_utils import run_bass_kernel_spmd
from concourse.masks import make_identity

# Initialize the PJRT backend immediately: the axon client handshake is
# flaky when the first device access happens long after process start.
try:
    import jax as _jax

    _jax.devices()
except Exception:
    pass

F32 = mybir.dt.float32
F32R = mybir.dt.float32r
BF16 = mybir.dt.bfloat16

B = 2
C = 64
SP = (16, 16, 16)
N = 4096
HEADS = 4
DH = 32
HID = HEADS * DH
EPS = 1e-12
NC128 = N // 128          # 32 token chunks of 128
NIB = N // 512            # 8 i-blocks of 512 (ATTN_NIB overrides for bisection)
W_EXP = 1536              # free-width of one exp op (3 psum banks)

ActF = mybir.ActivationFunctionType


def _cfg():
    return dict(
        qk_pack=int(os.environ.get("ATTN_QK_PACK", "4")),
        pv_groups=int(os.environ.get("ATTN_PV_GROUPS", "2")),
        mm_dt=os.environ.get("ATTN_MM_DT", "bf16"),
    )


def build_nc(qk_pack=4, pv_groups=2, mm_dt="bf16"):
    """Build the single-core Bass program (same program on all 8 cores)."""
    assert qk_pack in (1, 4) and pv_groups in (1, 2)
    MDT = BF16 if mm_dt == "bf16" else F32

    def mm_ap(ap):
        # access-pattern dtype used for PE matmul operands
        if mm_dt == "f32r":
            return ap.bitcast(F32R)
        return ap

    nc = bacc.Bacc("TRN2", target_bir_lowering=False, debug=False)

    xb = nc.dram_tensor("xb", [C, N], F32, kind="ExternalInput")
    wqkv = nc.dram_tensor("wqkv", [C, 3 * DH], F32, kind="ExternalInput")
    woT = nc.dram_tensor("woT", [DH, C], F32, kind="ExternalInput")
    out_tok = nc.dram_tensor("out_tok", [N, C], F32, kind="ExternalOutput")

    with tile.TileContext(nc) as tc:
        _body(tc, nc, xb, wqkv, woT, out_tok, qk_pack, pv_groups, MDT, mm_ap)
    nc.compile()
    return nc


def _body(tc, nc, xb, wqkv, woT, out_tok, qk_pack, pv_groups, MDT, mm_ap):
    const = tc.alloc_tile_pool(name="const", bufs=1)
    big = tc.alloc_tile_pool(name="big", bufs=1)
    work = tc.alloc_tile_pool(name="work", bufs=3)
    outp = tc.alloc_tile_pool(name="outp", bufs=3)
    epool = tc.alloc_tile_pool(name="epool", bufs=13)
    ps_sim = tc.alloc_tile_pool(name="ps_sim", bufs=2, space="PSUM")
    ps_pv = tc.alloc_tile_pool(name="ps_pv", bufs=1, space="PSUM")

    # ---- constants ----
    id_t = const.tile([128, 128], MDT, name="id_t")
    make_identity(nc, id_t)
    id_f = const.tile([128, 128], F32, name="id_f")
    make_identity(nc, id_f)
    ones64 = const.tile([C, 1], F32, name="ones64")
    nc.gpsimd.memset(ones64, 1.0)

    # ---- inputs ----
    x_sb = big.tile([C, N], F32, name="x_sb")
    for i in range(4):
        nc.sync.dma_start(out=x_sb[:, ts(i, N // 4)], in_=xb[:, ts(i, N // 4)])
    wqkv_sb = const.tile([C, 3 * DH], F32, name="wqkv_sb")
    nc.sync.dma_start(out=wqkv_sb, in_=wqkv[:, :])
    woT_sb = const.tile([DH, C], F32, name="woT_sb")
    nc.sync.dma_start(out=woT_sb, in_=woT[:, :])

    # ---- norm: sumsq per token -> inv = 1/max(sqrt(ss), eps) ----
    ps_ss = ps_pv.tile([128, NC128], F32, tag="pv", name="ps_ss")
    for c in range(NC128):
        xsq = work.tile([C, 128], F32, tag="xsq")
        nc.vector.tensor_mul(xsq, x_sb[:, ts(c, 128)], x_sb[:, ts(c, 128)])
        nc.tensor.matmul(
            ps_ss[:, c : c + 1], xsq, ones64,
            start=(c == 0), stop=(c == NC128 - 1),
        )
    l2_sb = const.tile([128, NC128], F32, name="l2_sb")
    nc.scalar.activation(l2_sb, ps_ss, ActF.Sqrt)
    nc.vector.tensor_scalar_max(out=l2_sb, in0=l2_sb, scalar1=EPS)
    invT = const.tile([128, NC128], F32, name="invT")
    nc.vector.reciprocal(invT, l2_sb)

    # ---- qkv projection into token-major bf16, normalized ----
    # qkv_sb[:, c, 0:32]=q', 32:64=k', 64:96=v', 96=ones
    qkv_sb = big.tile([128, NC128, 3 * DH + 1], MDT, name="qkv_sb")
    nc.vector.memset(qkv_sb[:, :, 3 * DH : 3 * DH + 1], 1.0)

    q4 = big.tile([128 if qk_pack == 4 else DH, N], MDT, name="q4")
    k4 = big.tile(
        [128 if qk_pack == 4 else DH, N // qk_pack], MDT, name="k4"
    )

    for c4 in range(NC128 // 4):
        # transpose the whole [128t, 97] chunk at once: out partitions 97
        # (rounds to 128 -> no column tiling, which is illegal with
        # transpose mode on TRN2). Rows 0:32 = q row-layout, 32:64 = k.
        ps_qkT = ps_pv.tile([128, 512], MDT, tag="pv", name="ps_qkT")
        for l in range(4):
            c = 4 * c4 + l
            ps_qkv = ps_sim.tile([128, 3 * DH], F32, tag="sim", name="ps_qkv")
            nc.tensor.matmul(
                ps_qkv, x_sb[:, ts(c, 128)], wqkv_sb, start=True, stop=True
            )
            nc.vector.tensor_scalar_mul(
                out=qkv_sb[:, c, 0 : 3 * DH],
                in0=ps_qkv,
                scalar1=invT[:, c : c + 1],
            )
            nc.tensor.transpose(
                ps_qkT[0 : 3 * DH + 1, ts(l, 128)], qkv_sb[:, c, :], id_t
            )
        # q row-layout: strip 0
        nc.vector.tensor_copy(q4[0:DH, ts(c4, 512)], ps_qkT[0:DH, :])
        if qk_pack == 4:
            # k chunk 4*c4+l -> partition strip l, columns ts(c4, 128)
            for l in range(4):
                nc.scalar.copy(
                    k4[DH * l : DH * (l + 1), ts(c4, 128)],
                    ps_qkT[DH : 2 * DH, ts(l, 128)],
                )
        else:
            nc.scalar.copy(k4[0:DH, ts(c4, 512)], ps_qkT[DH : 2 * DH, :])

    if qk_pack == 4:
        # replicate q into partition strips 1..3 (strip r reads SBUF
        # partitions 32r..32r+31 during row-tiled QK matmuls). DVE copies:
        # bf16 all-SBUF hits the 4x perf mode; SBUF->SBUF DMA is the less
        # exercised path on HW.
        for r in range(1, 4):
            for h in range(2):
                nc.vector.tensor_copy(
                    q4[DH * r : DH * (r + 1), ts(h, N // 2)],
                    q4[0:DH, ts(h, N // 2)],
                )

    # ---- main attention loop ----
    groups = []
    jc = 0
    while jc < NC128:
        g = list(range(jc, min(jc + W_EXP // 512, NC128)))
        groups.append(g)
        jc += len(g)

    n_ib = int(os.environ.get("ATTN_NIB", str(NIB)))
    for ib in range(n_ib):
        e_tiles = {}
        for g in groups:
            ps = ps_sim.tile([128, 512 * len(g)], F32, tag="sim", name="ps_s")
            for l, j in enumerate(g):
                if qk_pack == 4:
                    st = DH * (j % 4)
                    lhsT = k4[st : st + DH, ts(j // 4, 128)]
                    rhs = q4[st : st + DH, ts(ib, 512)]
                    tp = (st, 0)
                else:
                    lhsT = k4[0:DH, ts(j, 128)]
                    rhs = q4[0:DH, ts(ib, 512)]
                    tp = None
                nc.tensor.matmul(
                    ps[:, ts(l, 512)], mm_ap(lhsT), mm_ap(rhs),
                    start=True, stop=True, tile_position=tp,
                )
            et = epool.tile([128, 512 * len(g)], MDT, tag="e", name="e_t")
            nc.scalar.activation(et, ps, ActF.Exp)
            for l, j in enumerate(g):
                e_tiles[j] = et[:, ts(l, 512)]

        pv = ps_pv.tile([128, 1024], F32, tag="pv", name="pv")
        ngrp = 2 if pv_groups == 2 else 1
        for j in range(NC128):
            grp = j % ngrp
            base = 64 * grp
            nc.tensor.matmul(
                pv[base : base + DH + 1, ts(grp, 512)],
                mm_ap(qkv_sb[:, j, 2 * DH : 3 * DH + 1]),
                mm_ap(e_tiles[j]),
                start=(j < ngrp),
                stop=(j >= NC128 - ngrp),
                tile_position=(0, base) if pv_groups == 2 else None,
            )
        num = outp.tile([DH, 512], F32, tag="num")
        # o65 rows 0:64 = w_out_h @ num (pre-division), row 64 = denominator
        o65 = outp.tile([C + 1, 512], F32, tag="o")
        if pv_groups == 2:
            # DVE/walrus allow only one PSUM input per tensor_tensor op:
            # stage group 1 through SBUF first.
            pv1 = outp.tile([DH + 1, 512], F32, tag="pv1")
            nc.vector.tensor_copy(pv1, pv[64 : 64 + DH + 1, 512:1024])
            nc.vector.tensor_add(num, pv[0:DH, 0:512], pv1[0:DH, :])
            nc.vector.tensor_add(
                o65[C : C + 1, :], pv[DH : DH + 1, 0:512], pv1[DH : DH + 1, :]
            )
        else:
            nc.vector.tensor_copy(num, pv[0:DH, 0:512])
            nc.vector.tensor_copy(o65[C : C + 1, :], pv[DH : DH + 1, 0:512])

        ps_o = ps_pv.tile([C, 512], F32, tag="pv", name="ps_o")
        nc.tensor.matmul(ps_o, woT_sb, num, start=True, stop=True)
        nc.vector.tensor_copy(o65[0:C, :], ps_o)

        for t in range(4):
            ps_t = ps_pv.tile([128, C + 1], F32, tag="pv", name="ps_t")
            nc.tensor.transpose(
                ps_t, o65[:, ts(t, 128)], id_f[0 : C + 1, 0 : C + 1]
            )
            sinv = outp.tile([128, 1], F32, tag="sinv")
            nc.vector.reciprocal(sinv, ps_t[:, C : C + 1])
            out_c = outp.tile([128, C], F32, tag="outc")
            nc.vector.tensor_scalar_mul(out=out_c, in0=ps_t[:, 0:C], scalar1=sinv)
            nc.sync.dma_start(
                out=out_tok[ts(4 * ib + t, 128), :], in_=out_c
            )

    for p in (ps_pv, ps_sim, epool, outp, work, big, const):
        p.release()


_NC_CACHE = {}


def _get_nc():
    key = tuple(sorted(_cfg().items()))
    if key not in _NC_CACHE:
        _NC_CACHE[key] = build_nc(**_cfg())
    return _NC_CACHE[key]


def make_in_maps(x, g, w_qkv, w_out):
    """Per-core inputs. Core id = 4*batch + head."""
    x = np.asarray(x, np.float32)
    g = np.asarray(g, np.float32).reshape(C)
    w_qkv = np.asarray(w_qkv, np.float32)
    w_out = np.asarray(w_out, np.float32)

    colscale = g * np.sqrt(C)            # folded into all of q,k,v
    wq = w_qkv[0:HID] * colscale[None, :] * (DH ** -0.5)
    wk = w_qkv[HID : 2 * HID] * colscale[None, :]
    wv = w_qkv[2 * HID : 3 * HID] * colscale[None, :]

    in_maps = []
    for b in range(B):
        xbv = np.ascontiguousarray(x[b].reshape(C, N))
        for h in range(HEADS):
            sl = slice(DH * h, DH * (h + 1))
            wqkv_core = np.ascontiguousarray(
                np.concatenate([wq[sl], wk[sl], wv[sl]], axis=0).T
            ).astype(np.float32)
            woT_core = np.ascontiguousarray(w_out[:, sl].T).astype(np.float32)
            in_maps.append(
                {"xb": xbv, "wqkv": wqkv_core, "woT": woT_core}
            )
    return in_maps


def kernel(x, g, w_qkv, w_out, b_out):
    nc = _get_nc()
    in_maps = make_in_maps(x, g, w_qkv, w_out)
    res = run_bass_kernel_spmd(nc, in_maps, core_ids=list(range(8)))
    outs = [r["out_tok"] for r in res.results]
    b_out = np.asarray(b_out, np.float32)
    full = np.empty((B, C) + SP, np.float32)
    for b in range(B):
        acc = outs[4 * b].astype(np.float32).copy()
        for h in range(1, HEADS):
            acc += outs[4 * b + h]
        full[b] = acc.T.reshape((C,) + SP) + b_out[:, None, None, None]
    return full


# revision 15
# speedup vs baseline: 1.0367x; 1.0367x over previous
"""Trainium2 Bass kernel for nn_Attention3D (RMSNorm3D + 1x1x1 QKV conv +
4-head non-flash attention over n=4096 tokens + 1x1x1 output conv).

Sharding: b*heads = 2*4 = 8 independent attention instances -> one per
NeuronCore. Each core computes its (batch, head) attention and the partial
output projection partial_h = w_out[:, head] @ attn_out_h, returned
token-major [4096, 64]. Host sums the 4 head partials per batch and adds
the bias.

Per-core pipeline (all shapes per core):
  x_b [64, 4096] --(x^2, ones-matmul)--> sumsq per token -> inv = 1/max(l2,eps)
  qkv chunks: lhsT=x[:,128-chunk] @ w_qkv' [64,96] -> [128t, 96] PSUM
     (w' has g*sqrt(C) folded; q also has dh^-0.5)
     * inv[t] per-partition -> qkv_sb bf16 [128, 32, 97] (col 96 = ones)
  qT/kT 128x32 blocks are PE-transposed into row-layout q4 / k4 [32, 4096]
  scores (transposed): simT[j, i-block] = k_chunk.T @ q  (K=dh=32)
  e = exp(simT) on ScalarE (psum -> sbuf, bf16, no max-subtraction needed:
      |sim| <= ~1.2 for this problem's data)
  PV: lhsT = [v | ones] [128j, 33] -> out [33, i] accumulated over j chunks;
      row 32 = softmax denominator. 2-way column tiling.
  out-proj: woT.T @ num -> [64, i] psum; PE-transpose 128-chunks ->
      [128t, 64], * (1/s)[t] -> out_tok [4096, 64] f32 -> HBM
"""

import os

import numpy as np

import concourse.bass as bass
import concourse.mybir as mybir
import concourse.tile as tile
from concourse import bacc
from concourse.bass import ts
from concourse.bass_utils import run_bass_kernel_spmd
from concourse.masks import make_identity

# Initialize the PJRT backend immediately: the axon client handshake is
# flaky when the first device access happens long after process start.
try:
    import jax as _jax

    _jax.devices()
except Exception:
    pass

F32 = mybir.dt.float32
F32R = mybir.dt.float32r
BF16 = mybir.dt.bfloat16

B = 2
C = 64
SP = (16, 16, 16)
N = 4096
HEADS = 4
DH = 32
HID = HEADS * DH
EPS = 1e-12
NC128 = N // 128          # 32 token chunks of 128
NIB = N // 512            # 8 i-blocks of 512 (ATTN_NIB overrides for bisection)
W_EXP = int(os.environ.get("ATTN_WEXP", "1024"))  # exp op width (psum-capped)

ActF = mybir.ActivationFunctionType


def _cfg():
    return dict(
        qk_pack=int(os.environ.get("ATTN_QK_PACK", "4")),
        pv_groups=int(os.environ.get("ATTN_PV_GROUPS", "2")),
        mm_dt=os.environ.get("ATTN_MM_DT", "bf16"),
    )


def build_nc(qk_pack=4, pv_groups=2, mm_dt="bf16"):
    """Build the single-core Bass program (same program on all 8 cores)."""
    assert qk_pack in (1, 4) and pv_groups in (1, 2)
    MDT = BF16 if mm_dt == "bf16" else F32

    def mm_ap(ap):
        # access-pattern dtype used for PE matmul operands
        if mm_dt == "f32r":
            return ap.bitcast(F32R)
        return ap

    nc = bacc.Bacc("TRN2", target_bir_lowering=False, debug=False)

    xb = nc.dram_tensor("xb", [C, N], F32, kind="ExternalInput")
    wqkv = nc.dram_tensor("wqkv", [C, 3 * DH], F32, kind="ExternalInput")
    woT = nc.dram_tensor("woT", [DH, C], F32, kind="ExternalInput")
    out_tok = nc.dram_tensor("out_tok", [N, C], F32, kind="ExternalOutput")

    with tile.TileContext(nc) as tc:
        _body(tc, nc, xb, wqkv, woT, out_tok, qk_pack, pv_groups, MDT, mm_ap)
    nc.compile()
    return nc


def _body(tc, nc, xb, wqkv, woT, out_tok, qk_pack, pv_groups, MDT, mm_ap):
    const = tc.alloc_tile_pool(name="const", bufs=1)
    big = tc.alloc_tile_pool(name="big", bufs=1)
    work = tc.alloc_tile_pool(name="work", bufs=3)
    outp = tc.alloc_tile_pool(name="outp", bufs=3)
    epool = tc.alloc_tile_pool(name="epool", bufs=int(os.environ.get("ATTN_EBUFS", "13")))
    ps_sim = tc.alloc_tile_pool(
        name="ps_sim", bufs=(2 if W_EXP >= 1536 else 3), space="PSUM"
    )
    ps_pv = tc.alloc_tile_pool(name="ps_pv", bufs=1, space="PSUM")

    # ---- constants ----
    id_t = const.tile([128, 128], MDT, name="id_t")
    make_identity(nc, id_t)
    id_f = const.tile([128, 128], F32, name="id_f")
    make_identity(nc, id_f)
    ones64 = const.tile([C, 1], F32, name="ones64")
    nc.gpsimd.memset(ones64, 1.0)

    # ---- inputs ----
    x_sb = big.tile([C, N], F32, name="x_sb")
    for i in range(4):
        nc.sync.dma_start(out=x_sb[:, ts(i, N // 4)], in_=xb[:, ts(i, N // 4)])
    wqkv_sb = const.tile([C, 3 * DH], F32, name="wqkv_sb")
    nc.sync.dma_start(out=wqkv_sb, in_=wqkv[:, :])
    woT_sb = const.tile([DH, C], F32, name="woT_sb")
    nc.sync.dma_start(out=woT_sb, in_=woT[:, :])

    # ---- norm: sumsq per token -> inv = 1/max(sqrt(ss), eps) ----
    ps_ss = ps_pv.tile([128, NC128], F32, tag="pv", name="ps_ss")
    for c in range(NC128):
        xsq = work.tile([C, 128], F32, tag="xsq")
        nc.gpsimd.tensor_mul(xsq, x_sb[:, ts(c, 128)], x_sb[:, ts(c, 128)])
        nc.tensor.matmul(
            ps_ss[:, c : c + 1], xsq, ones64,
            start=(c == 0), stop=(c == NC128 - 1),
        )
    l2_sb = const.tile([128, NC128], F32, name="l2_sb")
    nc.scalar.activation(l2_sb, ps_ss, ActF.Sqrt)
    nc.vector.tensor_scalar_max(out=l2_sb, in0=l2_sb, scalar1=EPS)
    invT = const.tile([128, NC128], F32, name="invT")
    nc.vector.reciprocal(invT, l2_sb)

    # ---- qkv projection into token-major bf16, normalized ----
    # qkv_sb[:, c, 0:32]=q', 32:64=k', 64:96=v', 96=ones
    qkv_sb = big.tile([128, NC128, 3 * DH + 1], MDT, name="qkv_sb")
    nc.vector.memset(qkv_sb[:, :, 3 * DH : 3 * DH + 1], 1.0)

    q4 = big.tile([128 if qk_pack == 4 else DH, N], MDT, name="q4")
    k4 = big.tile(
        [128 if qk_pack == 4 else DH, N // qk_pack], MDT, name="k4"
    )

    for c4 in range(NC128 // 4):
        # transpose the whole [128t, 97] chunk at once: out partitions 97
        # (rounds to 128 -> no column tiling, which is illegal with
        # transpose mode on TRN2). Rows 0:32 = q row-layout, 32:64 = k.
        ps_qkT = ps_pv.tile([128, 512], MDT, tag="pv", name="ps_qkT")
        for l in range(4):
            c = 4 * c4 + l
            ps_qkv = ps_sim.tile([128, 3 * DH], F32, tag="sim", name="ps_qkv")
            nc.tensor.matmul(
                ps_qkv, x_sb[:, ts(c, 128)], wqkv_sb, start=True, stop=True
            )
            nc.vector.tensor_scalar_mul(
                out=qkv_sb[:, c, 0 : 3 * DH],
                in0=ps_qkv,
                scalar1=invT[:, c : c + 1],
            )
            nc.tensor.transpose(
                ps_qkT[0 : 3 * DH + 1, ts(l, 128)], qkv_sb[:, c, :], id_t
            )
        # q row-layout: strip 0
        nc.vector.tensor_copy(q4[0:DH, ts(c4, 512)], ps_qkT[0:DH, :])
        if qk_pack == 4:
            # k chunk 4*c4+l -> partition strip l, columns ts(c4, 128)
            for l in range(4):
                nc.vector.tensor_copy(
                    k4[DH * l : DH * (l + 1), ts(c4, 128)],
                    ps_qkT[DH : 2 * DH, ts(l, 128)],
                )
        else:
            nc.vector.tensor_copy(k4[0:DH, ts(c4, 512)], ps_qkT[DH : 2 * DH, :])

    if qk_pack == 4:
        # replicate q into partition strips 1..3 (strip r reads SBUF
        # partitions 32r..32r+31 during row-tiled QK matmuls). DVE copies:
        # bf16 all-SBUF hits the 4x perf mode; SBUF->SBUF DMA is the less
        # exercised path on HW.
        for r in range(1, 4):
            for h in range(2):
                nc.vector.tensor_copy(
                    q4[DH * r : DH * (r + 1), ts(h, N // 2)],
                    q4[0:DH, ts(h, N // 2)],
                )

    # ---- main attention loop ----
    groups = []
    jc = 0
    while jc < NC128:
        g = list(range(jc, min(jc + W_EXP // 512, NC128)))
        groups.append(g)
        jc += len(g)

    n_ib = int(os.environ.get("ATTN_NIB", str(NIB)))
    for ib in range(n_ib):
        e_tiles = {}
        for g in groups:
            ps = ps_sim.tile([128, 512 * len(g)], F32, tag="sim", name="ps_s")
            for l, j in enumerate(g):
                if qk_pack == 4:
                    st = DH * (j % 4)
                    lhsT = k4[st : st + DH, ts(j // 4, 128)]
                    rhs = q4[st : st + DH, ts(ib, 512)]
                    tp = (st, 0)
                else:
                    lhsT = k4[0:DH, ts(j, 128)]
                    rhs = q4[0:DH, ts(ib, 512)]
                    tp = None
                nc.tensor.matmul(
                    ps[:, ts(l, 512)], mm_ap(lhsT), mm_ap(rhs),
                    start=True, stop=True, tile_position=tp,
                )
            et = epool.tile([128, 512 * len(g)], MDT, tag="e", name="e_t")
            nc.scalar.activation(et, ps, ActF.Exp)
            for l, j in enumerate(g):
                e_tiles[j] = et[:, ts(l, 512)]

        pv = ps_pv.tile([128, 1024], F32, tag="pv", name="pv")
        ngrp = 2 if pv_groups == 2 else 1
        for j in range(NC128):
            grp = j % ngrp
            base = 64 * grp
            nc.tensor.matmul(
                pv[base : base + DH + 1, ts(grp, 512)],
                mm_ap(qkv_sb[:, j, 2 * DH : 3 * DH + 1]),
                mm_ap(e_tiles[j]),
                start=(j < ngrp),
                stop=(j >= NC128 - ngrp),
                tile_position=(0, base) if pv_groups == 2 else None,
            )
        num = outp.tile([DH, 512], F32, tag="num")
        # o65 rows 0:64 = w_out_h @ num (pre-division), row 64 = denominator
        o65 = outp.tile([C + 1, 512], F32, tag="o")
        if pv_groups == 2:
            # DVE/walrus allow only one PSUM input per tensor_tensor op:
            # stage group 1 through SBUF first.
            pv1 = outp.tile([DH + 1, 512], F32, tag="pv1")
            nc.vector.tensor_copy(pv1, pv[64 : 64 + DH + 1, 512:1024])
            nc.vector.tensor_add(num, pv[0:DH, 0:512], pv1[0:DH, :])
            nc.vector.tensor_add(
                o65[C : C + 1, :], pv[DH : DH + 1, 0:512], pv1[DH : DH + 1, :]
            )
        else:
            nc.vector.tensor_copy(num, pv[0:DH, 0:512])
            nc.vector.tensor_copy(o65[C : C + 1, :], pv[DH : DH + 1, 0:512])

        ps_o = ps_pv.tile([C, 512], F32, tag="pv", name="ps_o")
        nc.tensor.matmul(ps_o, woT_sb, num, start=True, stop=True)
        nc.vector.tensor_copy(o65[0:C, :], ps_o)

        for t in range(4):
            ps_t = ps_pv.tile([128, C + 1], F32, tag="pv", name="ps_t")
            nc.tensor.transpose(
                ps_t, o65[:, ts(t, 128)], id_f[0 : C + 1, 0 : C + 1]
            )
            sinv = outp.tile([128, 1], F32, tag="sinv")
            nc.vector.reciprocal(sinv, ps_t[:, C : C + 1])
            out_c = outp.tile([128, C], F32, tag="outc")
            nc.vector.tensor_scalar_mul(out=out_c, in0=ps_t[:, 0:C], scalar1=sinv)
            nc.sync.dma_start(
                out=out_tok[ts(4 * ib + t, 128), :], in_=out_c
            )

    for p in (ps_pv, ps_sim, epool, outp, work, big, const):
        p.release()


_NC_CACHE = {}


def _get_nc():
    key = tuple(sorted(_cfg().items()))
    if key not in _NC_CACHE:
        _NC_CACHE[key] = build_nc(**_cfg())
    return _NC_CACHE[key]


def make_in_maps(x, g, w_qkv, w_out):
    """Per-core inputs. Core id = 4*batch + head."""
    x = np.asarray(x, np.float32)
    g = np.asarray(g, np.float32).reshape(C)
    w_qkv = np.asarray(w_qkv, np.float32)
    w_out = np.asarray(w_out, np.float32)

    colscale = g * np.sqrt(C)            # folded into all of q,k,v
    wq = w_qkv[0:HID] * colscale[None, :] * (DH ** -0.5)
    wk = w_qkv[HID : 2 * HID] * colscale[None, :]
    wv = w_qkv[2 * HID : 3 * HID] * colscale[None, :]

    in_maps = []
    for b in range(B):
        xbv = np.ascontiguousarray(x[b].reshape(C, N))
        for h in range(HEADS):
            sl = slice(DH * h, DH * (h + 1))
            wqkv_core = np.ascontiguousarray(
                np.concatenate([wq[sl], wk[sl], wv[sl]], axis=0).T
            ).astype(np.float32)
            woT_core = np.ascontiguousarray(w_out[:, sl].T).astype(np.float32)
            in_maps.append(
                {"xb": xbv, "wqkv": wqkv_core, "woT": woT_core}
            )
    return in_maps


def kernel(x, g, w_qkv, w_out, b_out):
    nc = _get_nc()
    in_maps = make_in_maps(x, g, w_qkv, w_out)
    res = run_bass_kernel_spmd(nc, in_maps, core_ids=list(range(8)))
    outs = [r["out_tok"] for r in res.results]
    b_out = np.asarray(b_out, np.float32)
    full = np.empty((B, C) + SP, np.float32)
    for b in range(B):
        acc = outs[4 * b].astype(np.float32).copy()
        for h in range(1, HEADS):
            acc += outs[4 * b + h]
        full[b] = acc.T.reshape((C,) + SP) + b_out[:, None, None, None]
    return full
